# revision 2
# baseline (speedup 1.0000x reference)
"""Trainium2 Bass kernel for nn_CrossAttention — 4-core variant.

Sharding: 4 cores = 2 batches x 2 query-halves, fully SPMD, no collectives.
Each core handles 1024 queries against its batch's full 4096 keys, all 8
heads (K/V projection is 2x redundant per batch; queries are the serial
bottleneck via softmax-exp on the ACT engine, so splitting them halves the
critical path).  Measured against 2-core (1 batch/core) and 8-core
variants on HW, 4 cores minimizes per-call wall time.

Rationale (measured): the metric is per-call wall time through the axon
tunnel = ~82 ms fixed round-trip + ~150 us/extra core + ~10 us/(arg x core)
+ partially-visible kernel exec.  So: 2 cores instead of 8, ONE packed
input blob + ONE output blob per core instead of 13+2 tensors, and kernel
exec minimized.

Numerics: bf16 everywhere on the matmul path (f32 matmuls cost 4
cycles/row on TRN2, f32r needs >=256 moving cols, bf16 is always 1;
bf16 transposes are 2x cheaper than f32).  Kept f32: LN stats/outputs
(qin precision), the residual add, all PSUM accumulation, softmax
denominators + normalization, and the final W_o projection (f32r).
Logits are ~N(0, 0.2) so exp without max-subtraction is safe.
Weights/xT are packed as bf16 bit-pairs inside the f32 input blob by the
host (make_in_maps), so no on-chip conversion is needed.

Key implementation facts (hardware-validated in the 8-core ancestor):
- HARD RULE: every matmul operand/output sits at partition base 0; all
  64-partition matmuls pass tile_position=(0,0).
- Activations kept feature-on-partitions (transposed) so projections use
  natural-layout weights as stationary operand.
- Softmax denominators come free from a ones-column appended to V (row 64
  of the [65, q] AV output); reciprocals broadcast across partitions with
  a ones-column matmul.
"""

import os

import numpy as np

import concourse.bacc as bacc
import concourse.bass as bass
import concourse.mybir as mybir
import concourse.tile as tile
from concourse.bass_utils import run_bass_kernel_spmd
from concourse.masks import make_identity

F32 = mybir.dt.float32
F32R = mybir.dt.float32r
BF16 = mybir.dt.bfloat16
AF = mybir.ActivationFunctionType
ALU = mybir.AluOpType

# copy-engine assignment, tunable for sim A/B ("act" | "dve" | "pool")
CP = {"aT": "act", "vloc": "act", "qtkt": "act", "tpose": "act"}

# cross-attn exp: every DVE_EVERY-th key-tile group is evaluated on DVE as a
# degree-5 minimax polynomial (rel err 3e-3 over the full logit range
# |s*SCALE| <= 2.03 measured on the reference data); the rest use the ACT
# engine's native Exp.  Spreads the serial softmax-exp bottleneck over two
# engines.  0 disables the DVE path.
DVE_EVERY = int(os.environ.get("KDVE_EVERY", "0"))
# exp(s*SCALE) ~= PK*((((((s+PB5)*s+PB4)*s+PB3)*s+PB2)*s) + PC0 on s in +-16.3
PK = 2.2181380904560224e-07
PB5 = 53.89427961879931
PB4 = 1549.0252299577621
PB3 = 34768.17569235127
PB2 = 558482.4766286106
PC0 = 0.9996227667008071


def _copy(nc, eng, out, in_):
    if eng == "act":
        nc.scalar.activation(out=out, in_=in_, func=AF.Copy)
    elif eng == "dve":
        nc.vector.tensor_copy(out, in_)
    else:
        nc.gpsimd.tensor_copy(out, in_)


H, DH, CHUNK = 8, 64, 64
DIM = 512
INNER = 512
EPS = 1e-5
SCALE = DH ** -0.5

N_CORES = int(os.environ.get("KNCORES", "4"))
NQ = 4096 // N_CORES  # queries per core
NKT = 4096       # keys per core
NF = DIM // 128  # feature tiles (4)
NB = NQ // 512   # phase-1 query blocks (4)
NT = 4           # 128-row tiles per 512-row block
NKTT = NKT // 128  # key tiles (32)
XCHUNK = 512     # xT streaming chunk (keys)

# -------- input blob layout (flat f32 words, per core) --------
# bf16 regions store 2 bf16 values per f32 word (host packs bit patterns).
SZ_QX = NQ * DIM
SZ_XT_W = DIM * NKT // 2          # bf16
SZ_WQKV_W = DIM * 3 * INNER // 2  # bf16
SZ_WSQ_W = DIM * DIM // 2         # bf16
SZ_WKV_W = DIM * 2 * INNER // 2   # bf16
SZ_WO = INNER * DIM               # f32
OFF_QX = 0
OFF_XT = OFF_QX + SZ_QX
OFF_WQKV = OFF_XT + SZ_XT_W
OFF_WAO = OFF_WQKV + SZ_WQKV_W
OFF_WQ = OFF_WAO + SZ_WSQ_W
OFF_WKV = OFF_WQ + SZ_WSQ_W
OFF_WO = OFF_WKV + SZ_WKV_W
OFF_LN1G = OFF_WO + SZ_WO
OFF_LN1B = OFF_LN1G + DIM
OFF_LN2G = OFF_LN1B + DIM
OFF_LN2B = OFF_LN2G + DIM
OFF_BAO = OFF_LN2B + DIM
OFF_BO = OFF_BAO + DIM
TOTAL_IN = OFF_BO + DIM

# -------- output blob layout (flat f32, per core) --------
OFF_QIN = 0
OFF_OUT = SZ_QX          # outT [DIM, NQ]
TOTAL_OUT = 2 * SZ_QX


def _view(blob_ap, off, rows, cols, dtype=None):
    """Row-major [rows, cols(f32-words)] view at word offset `off`; optional
    bitcast (BF16 doubles the column count)."""
    ap = bass.AP(tensor=blob_ap.tensor, offset=off, ap=[[cols, rows], [1, cols]])
    return ap.bitcast(dtype) if dtype is not None else ap


def _bcast(blob_ap, off, n, parts):
    """[n] at offset -> [parts, n] partition-broadcast AP."""
    return bass.AP(tensor=blob_ap.tensor, offset=off, ap=[[0, parts], [1, n]])


def _layernorm(nc, pool, x_tiles, g_bc, b_bc, eps_tile, prefix, out_dtype=F32):
    out_tiles = []
    for tt in range(len(x_tiles)):
        x = x_tiles[tt]
        stats = pool.tile([128, 6], F32, name="ln_stats", tag="ln_stats")
        nc.vector.bn_stats(out=stats, in_=x)
        mv = pool.tile([128, 2], F32, name="ln_mv", tag="ln_mv")
        nc.vector.bn_aggr(out=mv, in_=stats)
        nc.scalar.activation(out=mv[:, 1:2], in_=mv[:, 1:2], func=AF.Sqrt,
                             bias=eps_tile, scale=1.0)
        nc.vector.reciprocal(out=mv[:, 1:2], in_=mv[:, 1:2])
        y = pool.tile([128, DIM], out_dtype, name=f"{prefix}{tt}", tag=f"{prefix}{tt}",
                      bufs=1)
        nc.vector.tensor_scalar(out=y, in0=x, scalar1=mv[:, 0:1], scalar2=mv[:, 1:2],
                                op0=ALU.subtract, op1=ALU.mult)
        nc.vector.tensor_tensor(out=y, in0=y, in1=g_bc, op=ALU.mult)
        nc.vector.tensor_tensor(out=y, in0=y, in1=b_bc, op=ALU.add)
        out_tiles.append(y)
    return out_tiles


def _transpose_to(nc, ps_pool, ident, src_tiles, dst_all, dst_col0):
    """dst_all[:, ft, dst_col0 + tt*128 : +128] = src[tt][:, ft*128:+128].T

    All NF transposes of one src tile land in one [128, NF, 128] PSUM tile
    and leave with ONE ACT copy (free-size 512) instead of four (ACT
    per-inst init ~370 ns is the dominant cost of small copies)."""
    for tt in range(len(src_tiles)):
        tp = ps_pool.tile([128, NF, 128], BF16, name="tposer", tag="tposer")
        for ft in range(NF):
            nc.tensor.transpose(tp[:, ft, :], src_tiles[tt][:, ft * 128:(ft + 1) * 128],
                                ident)
        c0 = dst_col0 + tt * 128
        _copy(nc, CP["tpose"], dst_all[:, :, c0:c0 + 128], tp)


def build_nc():
    nc = bacc.Bacc(None, target_bir_lowering=False)

    blob_d = nc.dram_tensor("blob", [TOTAL_IN], F32, kind="ExternalInput")
    out_d = nc.dram_tensor("res", [TOTAL_OUT], F32, kind="ExternalOutput")
    blob = blob_d[:]
    res = out_d[:]

    qx_v = _view(blob, OFF_QX, NQ, DIM)
    xT_v = _view(blob, OFF_XT, DIM, NKT // 2, BF16)
    Wqkv_v = _view(blob, OFF_WQKV, DIM, 3 * INNER // 2, BF16)
    Wao_v = _view(blob, OFF_WAO, INNER, DIM // 2, BF16)
    Wq_v = _view(blob, OFF_WQ, DIM, INNER // 2, BF16)
    Wkv_v = _view(blob, OFF_WKV, DIM, 2 * INNER // 2, BF16)
    Wo_v = _view(blob, OFF_WO, INNER, DIM, F32R)
    qin_v = _view(res, OFF_QIN, NQ, DIM)
    outT_v = _view(res, OFF_OUT, DIM, NQ)

    with tile.TileContext(nc) as tc:
        with tc.tile_pool(name="singles", bufs=1) as singles, \
             tc.tile_pool(name="persist", bufs=1) as persist:

            ident = singles.tile([128, 128], BF16)
            make_identity(nc, ident)
            eps_t = singles.tile([128, 1], F32)
            nc.vector.memset(eps_t, EPS)
            ones_t = singles.tile([1, 64], F32)
            nc.vector.memset(ones_t, 1.0)
            ones8 = singles.tile([128, 8, 1], BF16)
            nc.vector.memset(ones8, 1.0)

            g1 = singles.tile([128, DIM], F32)
            b1 = singles.tile([128, DIM], F32)
            g2 = singles.tile([128, DIM], F32)
            b2 = singles.tile([128, DIM], F32)
            bao_bc = singles.tile([128, DIM], F32)
            nc.gpsimd.dma_start(out=g1, in_=_bcast(blob, OFF_LN1G, DIM, 128))
            nc.gpsimd.dma_start(out=b1, in_=_bcast(blob, OFF_LN1B, DIM, 128))
            nc.gpsimd.dma_start(out=g2, in_=_bcast(blob, OFF_LN2G, DIM, 128))
            nc.gpsimd.dma_start(out=b2, in_=_bcast(blob, OFF_LN2B, DIM, 128))
            nc.gpsimd.dma_start(out=bao_bc, in_=_bcast(blob, OFF_BAO, DIM, 128))
            # bo as per-partition columns: bo[m*128 + p] -> bo_col[p, m]
            bo_col = singles.tile([128, NF], F32)
            nc.gpsimd.dma_start(
                out=bo_col,
                in_=bass.AP(tensor=blob.tensor, offset=OFF_BO,
                            ap=[[1, 128], [128, NF]]))

            # qcT survives into cross-attn (per-head, base-0), bf16
            qcT = [persist.tile([64, NQ], BF16, name=f"qcTh{h}", tag=f"qcTh{h}")
                   for h in range(H)]

            # =================== PHASE 1 (4 query blocks) ===================
            with tc.tile_pool(name="p1", bufs=1) as p1, \
                 tc.tile_pool(name="p1w", bufs=4) as p1w, \
                 tc.tile_pool(name="psT", bufs=2, space="PSUM") as psT, \
                 tc.tile_pool(name="psMM", bufs=2, space="PSUM") as psMM, \
                 tc.tile_pool(name="ps1", bufs=1, space="PSUM") as ps1:

                # qx DMAs first: LN1 is the head of the critical chain and
                # must not queue behind 5 MB of weight loads.
                qx_blk = []
                for qb in range(NB):
                    r0 = qb * 512
                    qx_t = []
                    for tt in range(NT):
                        x = p1.tile([128, DIM], F32, name=f"qx{qb}_{tt}",
                                    tag=f"qx{qb}_{tt}")
                        nc.sync.dma_start(
                            out=x, in_=qx_v[r0 + tt * 128:r0 + (tt + 1) * 128, :])
                        qx_t.append(x)
                    qx_blk.append(qx_t)
                Wqkv_sb = []
                Wao_sb = []
                Wq_sb = []
                for ft in range(NF):
                    w = p1.tile([128, 3 * INNER], BF16, name=f"wqkv{ft}", tag=f"wqkv{ft}")
                    nc.sync.dma_start(out=w, in_=Wqkv_v[ft * 128:(ft + 1) * 128, :])
                    Wqkv_sb.append(w)
                    w = p1.tile([128, DIM], BF16, name=f"wao{ft}", tag=f"wao{ft}")
                    nc.sync.dma_start(out=w, in_=Wao_v[ft * 128:(ft + 1) * 128, :])
                    Wao_sb.append(w)
                    w = p1.tile([128, INNER], BF16, name=f"wq{ft}", tag=f"wq{ft}")
                    nc.sync.dma_start(out=w, in_=Wq_v[ft * 128:(ft + 1) * 128, :])
                    Wq_sb.append(w)

                # ---- stage A (all blocks): LN1 -> lnT -> q/k/v projections.
                # Emitting A for every block before any block's local attention
                # software-pipelines phase 1: block b+1's dense LN/transpose/
                # projection work overlaps block b's serial attention chain.
                ablk = []
                for qb in range(NB):
                    r0 = qb * 512
                    qx_t = qx_blk[qb]

                    ln1 = _layernorm(nc, p1w, qx_t, g1, b1, eps_t, 'ln1_',
                                     out_dtype=BF16)

                    # ---- B. transpose -> lnT (bf16, one [128, NF, 512] tile)
                    lnT_a = p1.tile([128, NF, 512], BF16, name="lnT", tag="lnT", bufs=2)
                    _transpose_to(nc, psT, ident, ln1, lnT_a, 0)
                    lnT = [lnT_a[:, ft, :] for ft in range(NF)]

                    # ---- C. qkv projections
                    qT = [p1.tile([64, 512], BF16, name=f"qTh{h}", tag=f"qTh{h}",
                               bufs=2) for h in range(H)]
                    kT = [p1.tile([64, 512], BF16, name=f"kTh{h}", tag=f"kTh{h}",
                               bufs=2) for h in range(H)]
                    for m in range(8):  # 4 q tiles + 4 k tiles (transposed outputs)
                        ps = psMM.tile([128, 512], F32, name="proj_ps", tag="proj_ps")
                        for ft in range(NF):
                            nc.tensor.matmul(ps[:, :],
                                             Wqkv_sb[ft][:, m * 128:(m + 1) * 128],
                                             lnT[ft][:, :],
                                             start=(ft == 0), stop=(ft == NF - 1))
                        dst = qT if m < 4 else kT
                        mm = m % 4
                        _copy(nc, CP["qtkt"], dst[2 * mm], ps[0:64, :])
                        _copy(nc, CP["qtkt"], dst[2 * mm + 1], ps[64:128, :])
                    v_loc = [p1.tile([64, INNER], BF16, name=f"vloc{c}", tag=f"vloc{c}",
                                bufs=2) for c in range(8)]
                    for tt in range(NT):
                        ps = psMM.tile([128, INNER], F32, name="proj_ps", tag="proj_ps")
                        for ft in range(NF):
                            nc.tensor.matmul(ps[:, :],
                                             lnT[ft][:, tt * 128:(tt + 1) * 128],
                                             Wqkv_sb[ft][:, 2 * INNER:3 * INNER],
                                             start=(ft == 0), stop=(ft == NF - 1))
                        _copy(nc, CP["vloc"], v_loc[2 * tt], ps[0:64, :])
                        _copy(nc, CP["vloc"], v_loc[2 * tt + 1], ps[64:128, :])
                    ablk.append((r0, qx_t, qT, kT, v_loc))

                # ---- stages B+C per block
                for r0, qx_t, qT, kT, v_loc in ablk:
                    # ---- D. chunked local attention -> oT_local
                    oT_local = [p1.tile([128, 512], BF16, name=f"oTl{m}", tag=f"oTl{m}",
                                   bufs=2) for m in range(NF)]
                    for cp in range(NT):
                        s_ps = [ps1.tile([64, H, CHUNK], F32, name=f"s_loc{c01}",
                                         tag=f"s_loc{c01}") for c01 in range(2)]
                        for h in range(H):
                            for c01 in range(2):
                                c0 = cp * 128 + c01 * 64
                                nc.tensor.matmul(s_ps[c01][:, h, :],
                                                 qT[h][:, c0:c0 + 64],
                                                 kT[h][:, c0:c0 + 64],
                                                 start=True, stop=True,
                                                 tile_position=(0, 0))
                        a_sb = [None, None]
                        for c01 in range(2):
                            a = p1w.tile([64, H, CHUNK], BF16, name="a_loc",
                                         tag="a_loc")
                            nc.scalar.activation(out=a, in_=s_ps[c01], func=AF.Exp,
                                                 scale=SCALE)
                            sums = p1w.tile([64, H], F32, name="sums_loc",
                                            tag="sums_loc")
                            nc.vector.tensor_reduce(out=sums, in_=a,
                                                    axis=mybir.AxisListType.X,
                                                    op=ALU.add)
                            nc.vector.reciprocal(out=sums, in_=sums)
                            nc.vector.tensor_tensor(
                                out=a, in0=a, in1=sums.broadcast_to((64, H, CHUNK)),
                                op=ALU.mult)
                            a_sb[c01] = a
                        for h in range(H):
                            hp, hr = h // 2, (h % 2) * 64
                            av_ps = ps1.tile([64, 128], F32, name="av_loc",
                                             tag="av_loc", bufs=2)
                            for c01 in range(2):
                                aT_ps = psT.tile([64, 64], BF16, name="tposer",
                                                 tag="tposer")
                                nc.tensor.transpose(aT_ps[:, :], a_sb[c01][:, h, :],
                                                    ident[0:64, 0:64])
                                aT = p1w.tile([64, 64], BF16, name="aT_sb",
                                              tag="aT_sb")
                                _copy(nc, CP["aT"], aT, aT_ps)
                                vs = v_loc[cp * 2 + c01][:, h * 64:(h + 1) * 64]
                                nc.tensor.matmul(av_ps[:, c01 * 64:(c01 + 1) * 64],
                                                 vs, aT, start=True, stop=True,
                                                 tile_position=(0, 0))
                            nc.vector.tensor_copy(
                                oT_local[hp][hr:hr + 64, cp * 128:(cp + 1) * 128],
                                av_ps)

                    # ---- E. W_ao projection + bias + residual
                    ao = [p1.tile([128, DIM], F32, name=f"ao{tt}", tag=f"ao{tt}",
                               bufs=2) for tt in range(NT)]
                    for tt in range(NT):
                        ps = psMM.tile([128, 512], F32, name="proj_ps", tag="proj_ps")
                        for ft in range(NF):
                            nc.tensor.matmul(ps[:, :],
                                             oT_local[ft][:, tt * 128:(tt + 1) * 128],
                                             Wao_sb[ft][:, :],
                                             start=(ft == 0), stop=(ft == NF - 1))
                        nc.vector.tensor_tensor(out=ao[tt], in0=ps, in1=bao_bc,
                                                op=ALU.add)
                        nc.vector.tensor_tensor(out=ao[tt], in0=ao[tt], in1=qx_t[tt],
                                                op=ALU.add)

                    # ---- F. LN2 -> q_in (DMA out, f32) ; bf16 copy -> qinT
                    qin = _layernorm(nc, p1w, ao, g2, b2, eps_t, 'qin_')
                    qin_b = []
                    for tt in range(NT):
                        nc.sync.dma_start(
                            out=qin_v[r0 + tt * 128:r0 + (tt + 1) * 128, :],
                            in_=qin[tt])
                        yb = p1w.tile([128, DIM], BF16, name="qin_b", tag="qin_b")
                        nc.vector.tensor_copy(yb, qin[tt])
                        qin_b.append(yb)
                    qinT_a = p1.tile([128, NF, 512], BF16, name="qinT", tag="qinT", bufs=2)
                    _transpose_to(nc, psT, ident, qin_b, qinT_a, 0)
                    qinT = [qinT_a[:, ft, :] for ft in range(NF)]

                    # ---- G. W_q projection -> qcT columns (bf16, persists)
                    for m in range(NF):
                        ps = psMM.tile([128, 512], F32, name="proj_ps", tag="proj_ps")
                        for ft in range(NF):
                            nc.tensor.matmul(ps[:, :],
                                             Wq_sb[ft][:, m * 128:(m + 1) * 128],
                                             qinT[ft][:, :],
                                             start=(ft == 0), stop=(ft == NF - 1))
                        nc.vector.tensor_copy(qcT[2 * m][:, r0:r0 + 512], ps[0:64, :])
                        nc.vector.tensor_copy(qcT[2 * m + 1][:, r0:r0 + 512],
                                              ps[64:128, :])

            # =================== PHASE 2a: project all K/V ===================
            _PH = os.environ.get("KPH", "all")
            if _PH == "p1":
                with tc.tile_pool(name="dummy", bufs=1) as dummy:
                    for m in range(NF):
                        z = dummy.tile([64, 512], F32, name=f"z{m}", tag=f"z{m}")
                        nc.vector.tensor_copy(z, qcT[m][:, 0:1024].bitcast(F32))
                        nc.sync.dma_start(out=outT_v[m * 128:m * 128 + 64, 0:512],
                                          in_=z)
            if _PH != "p1":
              with tc.tile_pool(name="p2p", bufs=1) as p2p, \
                 tc.tile_pool(name="kc", bufs=1) as kc_pool, \
                 tc.tile_pool(name="vc", bufs=1) as vc_pool, \
                 tc.tile_pool(name="ot", bufs=1) as ot_pool:
                kcT = [kc_pool.tile([64, NKT], BF16, name=f"kcTh{h}", tag=f"kcTh{h}")
                       for h in range(H)]
                v_aug = [vc_pool.tile([128, H, 65], BF16, name=f"vaug{kt}",
                                      tag=f"vaug{kt}") for kt in range(NKTT)]
                # per-head unnormalized output + denominator row, accumulated in
                # SBUF across key chunks so attention interleaves with the K/V
                # projection stream (exp starts ~60 us earlier than with one
                # long PSUM accumulation after all projections).
                oT_sb = [ot_pool.tile([65, NQ], F32, name=f"oT{h}", tag=f"oT{h}")
                         for h in range(H)]
                KTL = XCHUNK // 128  # key tiles per chunk

                with tc.tile_pool(name="wkv", bufs=1) as wkv_pool, \
                     tc.tile_pool(name="xc", bufs=2) as xc_pool, \
                     tc.tile_pool(name="pa", bufs=4) as pa_pool, \
                     tc.tile_pool(name="ps_p", bufs=2, space="PSUM") as ps_p, \
                     tc.tile_pool(name="ps_s", bufs=2, space="PSUM") as ps_s, \
                     tc.tile_pool(name="ps_o", bufs=1, space="PSUM") as ps_o:
                    Wkv_sb = []
                    for ft in range(NF):
                        w = wkv_pool.tile([128, 2 * INNER], BF16, name=f"wkv{ft}",
                                          tag=f"wkv{ft}")
                        nc.sync.dma_start(out=w, in_=Wkv_v[ft * 128:(ft + 1) * 128, :])
                        Wkv_sb.append(w)
                    for chunk in range(NKT // XCHUNK):
                        k0 = chunk * XCHUNK
                        xTc = []
                        for ft in range(NF):
                            xt = xc_pool.tile([128, XCHUNK], BF16, name=f"xTc{ft}",
                                              tag=f"xTc{ft}")
                            nc.sync.dma_start(
                                out=xt,
                                in_=xT_v[ft * 128:(ft + 1) * 128, k0:k0 + XCHUNK])
                            xTc.append(xt)
                        # K^T -> kcT columns
                        for m in range(NF):
                            ps = ps_p.tile([128, XCHUNK], F32, name="proj_ps",
                                           tag="proj_ps")
                            for ft in range(NF):
                                nc.tensor.matmul(
                                    ps[:, :],
                                    Wkv_sb[ft][:, m * 128:(m + 1) * 128],
                                    xTc[ft][:, :],
                                    start=(ft == 0), stop=(ft == NF - 1))
                            nc.vector.tensor_copy(kcT[2 * m][:, k0:k0 + XCHUNK],
                                                  ps[0:64, :])
                            nc.vector.tensor_copy(kcT[2 * m + 1][:, k0:k0 + XCHUNK],
                                                  ps[64:128, :])
                        # V (natural) + ones column -> v_aug
                        for ktl in range(KTL):
                            kt = chunk * KTL + ktl
                            ps = ps_p.tile([128, INNER], F32, name="proj_ps",
                                           tag="proj_ps")
                            for ft in range(NF):
                                nc.tensor.matmul(
                                    ps[:, :],
                                    xTc[ft][:, ktl * 128:(ktl + 1) * 128],
                                    Wkv_sb[ft][:, INNER:2 * INNER],
                                    start=(ft == 0), stop=(ft == NF - 1))
                            nc.vector.tensor_copy(
                                v_aug[kt][:, :, 0:64],
                                ps[:, :].rearrange("p (h d) -> p h d", h=H))
                            nc.vector.tensor_copy(v_aug[kt][:, :, 64:65], ones8)
                        # ---- this chunk's attention contribution, all heads.
                        # Matmul outputs must fit one 2KB PSUM bank (512 f32
                        # cols), so the NQ queries are processed as NJ 512-col
                        # halves sharing one [128, NJ, 512] score tile; exp
                        # still covers the whole tile in one instruction.
                        NJ = NQ // 512
                        for h in range(H) if _PH != "p2a" else []:
                            o_ps = [ps_o.tile([65, 512], F32, name=f"o_ps{j}",
                                              tag=f"o_ps{j}") for j in range(NJ)]
                            for ktl in range(KTL):
                                kt = chunk * KTL + ktl
                                s_ps = ps_s.tile([128, NJ, 512], F32, name="s_ps",
                                                 tag="s_ps")
                                for j in range(NJ):
                                    nc.tensor.matmul(
                                        s_ps[:, j, :],
                                        kcT[h][:, kt * 128:(kt + 1) * 128],
                                        qcT[h][:, j * 512:(j + 1) * 512],
                                        start=True, stop=True,
                                        tile_position=(0, 0))
                                a_sb = pa_pool.tile([128, NJ, 512], BF16,
                                                    name="a_sb", tag="a_sb")
                                nc.scalar.activation(out=a_sb, in_=s_ps,
                                                     func=AF.Exp, scale=SCALE)
                                for j in range(NJ):
                                    nc.tensor.matmul(
                                        o_ps[j][:, :],
                                        v_aug[kt][:, h, :],
                                        a_sb[:, j, :],
                                        start=(ktl == 0),
                                        stop=(ktl == KTL - 1))
                            for j in range(NJ):
                                sl = slice(j * 512, (j + 1) * 512)
                                if chunk == 0:
                                    nc.vector.tensor_copy(oT_sb[h][:, sl], o_ps[j])
                                else:
                                    nc.vector.tensor_tensor(
                                        out=oT_sb[h][:, sl], in0=oT_sb[h][:, sl],
                                        in1=o_ps[j], op=ALU.add)

                # =================== normalize ===================
                if _PH == "p2a":
                    for m in range(NF):
                        z2 = p2p.tile([64, 512], F32, name=f"z2{m}", tag=f"z2{m}")
                        nc.vector.tensor_copy(z2, kcT[m][:, 0:1024].bitcast(F32))
                        nc.sync.dma_start(out=outT_v[m * 128:m * 128 + 64, 0:512],
                                          in_=z2)
                oT_norm = [p2p.tile([128, NQ], F32R, name=f"oTn{m}", tag=f"oTn{m}")
                           for m in range(NF)] if _PH != "p2a" else []
                with tc.tile_pool(name="fin", bufs=2) as fin, \
                     tc.tile_pool(name="ps_b", bufs=2, space="PSUM") as ps_b:
                    for h in range(H) if _PH != "p2a" else []:
                        hp, hr = h // 2, (h % 2) * 64
                        rec = fin.tile([1, NQ], F32, name="rec", tag="rec")
                        nc.vector.reciprocal(rec, oT_sb[h][64:65, :])
                        for j in range(NQ // 512):
                            sl = slice(j * 512, (j + 1) * 512)
                            bc_ps = ps_b.tile([64, 512], F32, name="bc_ps",
                                              tag="bc_ps")
                            nc.tensor.matmul(bc_ps[:, :], ones_t[0:1, :],
                                             rec[0:1, sl], start=True, stop=True)
                            bc_sb = fin.tile([64, 512], F32, name="bc_sb",
                                             tag="bc_sb")
                            nc.vector.tensor_copy(bc_sb, bc_ps)
                            nc.vector.tensor_tensor(
                                out=oT_norm[hp][hr:hr + 64, sl],
                                in0=oT_sb[h][0:64, sl], in1=bc_sb, op=ALU.mult)

                # =================== W_o projection ===================
                with tc.tile_pool(name="wo", bufs=1) as wo_pool, \
                     tc.tile_pool(name="fin2", bufs=2) as fin2, \
                     tc.tile_pool(name="ps_f", bufs=2, space="PSUM") as ps_f:
                    Wo_sb = []
                    for ft in range(NF):
                        w = wo_pool.tile([128, DIM], F32R, name=f"wo{ft}", tag=f"wo{ft}")
                        nc.sync.dma_start(out=w, in_=Wo_v[ft * 128:(ft + 1) * 128, :])
                        Wo_sb.append(w)
                    for m in range(NF) if _PH != "p2a" else []:
                        for qb in range(NB):
                            q0 = qb * 512
                            ps = ps_f.tile([128, 512], F32, name="out_ps", tag="out_ps")
                            for ft in range(NF):
                                nc.tensor.matmul(
                                    ps[:, :],
                                    Wo_sb[ft][:, m * 128:(m + 1) * 128],
                                    oT_norm[ft][:, q0:q0 + 512],
                                    start=(ft == 0), stop=(ft == NF - 1))
                            ot = fin2.tile([128, 512], F32, name="outT_sb",
                                           tag="outT_sb")
                            nc.vector.tensor_scalar(out=ot, in0=ps,
                                                    scalar1=bo_col[:, m:m + 1],
                                                    scalar2=None, op0=ALU.add)
                            nc.sync.dma_start(
                                out=outT_v[m * 128:(m + 1) * 128, q0:q0 + 512], in_=ot)

    nc.finalize()
    return nc


_NC_CACHE = {}


def _bf_pack(a):
    """f32 array -> bf16 bit patterns packed 2-per-f32-word (flat)."""
    import ml_dtypes
    bf = np.ascontiguousarray(a, np.float32).astype(ml_dtypes.bfloat16)
    return bf.ravel().view(np.float32)


def make_in_maps(x, q_x, ln1_g, ln1_b, W_qkv, W_ao, b_ao, ln2_g, ln2_b,
                 W_q, W_kv, W_o, b_o):
    """Pack full inputs into per-core blobs (query-rows sharded, keys full)."""
    f = lambda a: np.ascontiguousarray(a, np.float32).ravel()
    wtail = np.concatenate([
        _bf_pack(W_qkv), _bf_pack(W_ao), _bf_pack(W_q), _bf_pack(W_kv),
        f(W_o), f(ln1_g), f(ln1_b), f(ln2_g), f(ln2_b), f(b_ao), f(b_o)])
    halves = N_CORES // 2
    xp = [_bf_pack(np.asarray(x)[b].T) for b in range(2)]
    in_maps = []
    for c in range(N_CORES):
        b, hh = c // halves, c % halves
        blob = np.concatenate([f(q_x[b, hh * NQ:(hh + 1) * NQ]), xp[b], wtail])
        assert blob.size == TOTAL_IN, (blob.size, TOTAL_IN)
        in_maps.append({"blob": blob})
    return in_maps


def kernel(x, q_x, ln1_g, ln1_b, W_qkv, W_ao, b_ao, ln2_g, ln2_b,
           W_q, W_kv, W_o, b_o):
    if "nc" not in _NC_CACHE:
        _NC_CACHE["nc"] = build_nc()
    nc = _NC_CACHE["nc"]

    in_maps = make_in_maps(x, q_x, ln1_g, ln1_b, W_qkv, W_ao, b_ao,
                           ln2_g, ln2_b, W_q, W_kv, W_o, b_o)
    res = run_bass_kernel_spmd(nc, in_maps, core_ids=list(range(N_CORES)))

    B, NQ_, D = q_x.shape
    halves = N_CORES // 2
    out = np.empty((B, NQ_, D), np.float32)
    q_in = np.empty((B, NQ_, D), np.float32)
    for c in range(N_CORES):
        b, hh = c // halves, c % halves
        r = res.results[c]["res"]
        rows = slice(hh * NQ, (hh + 1) * NQ)
        q_in[b, rows] = r[OFF_QIN:OFF_QIN + SZ_QX].reshape(NQ, D)
        out[b, rows] = r[OFF_OUT:OFF_OUT + SZ_QX].reshape(D, NQ).T
    return (out, q_in)
